# revision 18
# baseline (speedup 1.0000x reference)
"""CrossModalMatchingNetwork Trainium2 kernel.

Full-input contract: kernel(**inputs) takes the unsharded numpy inputs and
returns the full [B, S, S] cosine-similarity output (float32).

Strategy: data-parallel over batch across 8 NeuronCores (2 batches/core).
Host-side prep transposes the big activations to [D, S] layout so the
contraction dim lands on SBUF partitions, casts them to bf16 (fp32 PSUM
accumulation), and replicates the small projection weights (pre-transposed
to [D, H]) to every core.

Per core, per batch (engine-balanced schedule):
  tT[h,s]  = sum_d WtT[d,h] * txtT[d,s] + bt[h]     (k-outer: 4 interleaved
  vT[h,s]  = sum_d WvT[d,h] * visT[d,s] + bv[h]      PSUM chains trickle-feed
                                                      from DMA at startup)
  tn2[s]   = sum_h tT[h,s]^2   (DVE squares+adds -> ones-col matmul row)
  tT      *= 1/tn   (DVE approx-recip row -> sqrt -> ones-row repl matmul;
                     DVE multiplies tT by the replicated PSUM directly)
  vn2      = per-column sums via tiny stationary=vss moving=ones matmuls,
             giving vn2 directly in COLUMN layout [s-block(P), i]
  dots     = vT.T @ tT~           (raw vT stationary; 1/tn already folded)
  out      = dots * (1/vn)[partition] applied during the PSUM->SBUF copy,
             alternating Scalar/DVE so neither queue backs up
Output is written bf16 (halves output DMA); host upcasts to float32.

DMA design: the Tile framework tracks dependencies per TILE (a reader waits
for all prior writers of the tile), so every 128-partition chunk is its own
tile, split further into 512-column halves; all heavyweight DMAs ride the
sync queue in exact compute need-order (wt, txt-h0 interleaved, txt-h1,
wv + vis-h0 interleaved, vis-h1, batch-1 inputs), which makes the startup
trickle-feed run at wire speed.
"""

import numpy as np
from contextlib import ExitStack

import concourse.bass as bass
import concourse.mybir as mybir
import concourse.tile as tile
from concourse import bacc
from concourse.bass import ds, ts

B, S, VD, TD, H = 16, 1024, 1024, 768, 512
NCORES = 8
BPC = B // NCORES  # batches per core
P = 128
FD = 512  # matmul moving-operand free dim (one PSUM bank of fp32)

F32 = mybir.dt.float32
BF16 = mybir.dt.bfloat16

AF = mybir.ActivationFunctionType


def build(bpc=BPC, s=S, vd=VD, td=TD, h=H):
    kv, kt, mh = vd // P, td // P, h // P
    ns, ms = s // FD, s // P  # 2 free-dim halves, 8 s-blocks of 128
    CT = BF16

    nc = bacc.Bacc("TRN2", target_bir_lowering=False)
    txtT = nc.dram_tensor("txtT", [bpc, td, s], CT, kind="ExternalInput")
    visT = nc.dram_tensor("visT", [bpc, vd, s], CT, kind="ExternalInput")
    wtT = nc.dram_tensor("wtT", [td, h], CT, kind="ExternalInput")
    wvT = nc.dram_tensor("wvT", [vd, h], CT, kind="ExternalInput")
    btp = nc.dram_tensor("btp", [P, mh], F32, kind="ExternalInput")
    bvp = nc.dram_tensor("bvp", [P, mh], F32, kind="ExternalInput")
    onesd = nc.dram_tensor("ones", [P, P], CT, kind="ExternalInput")
    out = nc.dram_tensor("out", [bpc, s, s], CT, kind="ExternalOutput")

    with (
        tile.TileContext(nc) as tc,
        ExitStack() as ctx,
        nc.allow_low_precision(reason="compute dtype is bf16 by design"),
    ):
        consts = ctx.enter_context(tc.tile_pool(name="consts", bufs=1))
        txt_pool = ctx.enter_context(tc.tile_pool(name="txt", bufs=2))
        vis_pool = ctx.enter_context(tc.tile_pool(name="vis", bufs=2))
        tt_pool = ctx.enter_context(tc.tile_pool(name="tt", bufs=1))
        vt_pool = ctx.enter_context(tc.tile_pool(name="vt", bufs=1))
        sq_pool = ctx.enter_context(tc.tile_pool(name="sq", bufs=1))
        row_pool = ctx.enter_context(tc.tile_pool(name="rows", bufs=2))
        col_pool = ctx.enter_context(tc.tile_pool(name="cols", bufs=1))
        out_pool = ctx.enter_context(tc.tile_pool(name="outs", bufs=3))
        ps_mm = ctx.enter_context(tc.tile_pool(name="ps_mm", bufs=6, space="PSUM"))
        ps_repl = ctx.enter_context(tc.tile_pool(name="ps_repl", bufs=2, space="PSUM"))

        # --- weights: per-chunk contiguous tiles; wt FIRST on the scalar
        # queue (per-queue transfers serialize, and wt0 gates the very first
        # matmul) running in parallel with txt's on the sync queue
        wt_k = []
        for k in range(kt):
            w = consts.tile([P, h], CT, name=f"wt{k}")
            nc.scalar.dma_start(w[:], wtT[ds(k * P, P), :])
            wt_k.append(w)
        wv_k = [consts.tile([P, h], CT, name=f"wv{k}") for k in range(kv)]

        # --- small constants after the weights (not needed until ~15us)
        bt_sb = consts.tile([P, mh], F32)
        nc.scalar.dma_start(bt_sb[:], btp[:, :])
        bv_sb = consts.tile([P, mh], F32)
        nc.scalar.dma_start(bv_sb[:], bvp[:, :])
        ones_sb = consts.tile([P, P], CT)
        nc.scalar.dma_start(ones_sb[:], onesd[:, :])
        ones_col = ones_sb[:, 0:1]
        ones_row = ones_sb[0:1, :]

        # PE warm-up while the first input DMAs are in flight, so HAM is at
        # full clock for real work; plus activation-table prewarm (Identity/
        # Sqrt/Copy) so no ACT_TABLE_LOAD stalls the scalar queue mid-kernel.
        warm_sb = consts.tile([P, P], CT)
        nc.vector.memset(warm_sb[:], 0.0)
        warm_f = consts.tile([1, 8], F32)
        nc.vector.memset(warm_f[:], 1.0)
        warm_ps = ps_repl.tile([P, FD], F32, tag="ps_repl")
        for _ in range(16):
            nc.tensor.matmul(warm_ps[:, 0:P], warm_sb[:], warm_sb[:])
        nc.scalar.activation(warm_sb[0:1, 0:8], warm_f[:], AF.Identity)
        nc.scalar.activation(warm_sb[0:1, 0:8], warm_f[:], AF.Sqrt)
        nc.scalar.activation(warm_sb[0:1, 0:8], warm_ps[0:1, 0:8], AF.Copy)

        def proj_mm(kk, w_tiles, x_tiles, n2):
            """k-outer matmul chains for one free-dim half; returns PSUM tiles.
            x_tiles[k] is a [P, s] chunk tile."""
            sl = ds(n2 * FD, FD)
            pvs = [
                ps_mm.tile([P, FD], F32, tag="ps_mm", name=f"pj{m}")
                for m in range(mh)
            ]
            for k in range(kk):
                for m in range(mh):
                    nc.tensor.matmul(
                        pvs[m][:],
                        w_tiles[k][:, ts(m, P)],
                        x_tiles[k][:, sl],
                        start=(k == 0),
                        stop=(k == kk - 1),
                    )
            return pvs

        def proj_act(pvs, b_sb, y_sb, n2):
            sl = ds(n2 * FD, FD)
            for m in range(mh):
                nc.scalar.activation(
                    y_sb[:, m, sl], pvs[m][:], AF.Identity, bias=b_sb[:, ds(m, 1)]
                )

        def squares(y_sb, ysq_sb, n2):
            sl = ds(n2 * FD, FD)
            nc.vector.tensor_mul(
                ysq_sb[:, :, sl], y_sb[:, :, sl], y_sb[:, :, sl]
            )

        def chunk_sum(ysq_sb, yss_sb, n2):
            sl = ds(n2 * FD, FD)
            nc.vector.tensor_add(yss_sb[:, sl], ysq_sb[:, 0, sl], ysq_sb[:, 1, sl])
            for m in range(2, mh):
                nc.vector.tensor_add(yss_sb[:, sl], yss_sb[:, sl], ysq_sb[:, m, sl])

        # --- input loads: per-chunk tiles (dependency tracking is per tile,
        # so a k-group's matmul waits only for its own chunk), one full-chunk
        # DMA each (queue issue cost is ~fixed per DMA), all on the sync
        # queue in exact compute need-order for BOTH batches up front.
        inq = [nc.sync, nc.gpsimd]
        txt_b, vis_b = [], []
        for b in range(bpc):
            txt_t = []
            for k in range(kt):
                t = txt_pool.tile([P, s], CT, name=f"tx{b}_{k}")
                nc.sync.dma_start(t[:], txtT[b, ds(k * P, P), :])
                txt_t.append(t)
            txt_b.append(txt_t)
            vis_t = []
            for k in range(kv):
                t = vis_pool.tile([P, s], CT, name=f"vi{b}_{k}")
                nc.sync.dma_start(t[:], visT[b, ds(k * P, P), :])
                if b == 0:  # wv chunk k rides right behind vis chunk k
                    nc.sync.dma_start(wv_k[k][:], wvT[ds(k * P, P), :])
                vis_t.append(t)
            vis_b.append(vis_t)

        for b in range(bpc):
            txt_t, vis_t = txt_b[b], vis_b[b]
            tt_sb = tt_pool.tile([P, mh, s], CT)
            vt_sb = vt_pool.tile([P, mh, s], CT)
            tsq_sb = sq_pool.tile([P, mh, s], CT, tag="tsq")
            vsq_sb = sq_pool.tile([P, mh, s], CT, tag="vsq")
            tss_sb = sq_pool.tile([P, s], CT, tag="tss")
            vss_sb = sq_pool.tile([P, s], CT, tag="vss")
            rvn_cols = col_pool.tile([P, ms], F32, tag="rvn")

            # --- proj-t (both halves) + squares + tn2 partial sums
            for n2 in range(ns):
                pts = proj_mm(kt, wt_k, txt_t, n2)
                proj_act(pts, bt_sb, tt_sb, n2)
                squares(tt_sb, tsq_sb, n2)
            for n2 in range(ns):
                chunk_sum(tsq_sb, tss_sb, n2)

            # --- proj-v first half (PE stays busy while the t-norm chain
            # runs on DVE/Scalar)
            pv0 = proj_mm(kv, wv_k, vis_t, 0)
            proj_act(pv0, bv_sb, vt_sb, 0)

            # --- t-norm chain: tn2 row -> approx 1/tn2 -> sqrt(-> 1/tn row)
            rrows, srows = [], []
            pn_t = []
            for n2 in range(ns):
                sl = ds(n2 * FD, FD)
                pn = ps_repl.tile([1, FD], F32, tag="ps_repl", name=f"pn{n2}")
                nc.tensor.matmul(pn[:], ones_col, tss_sb[:, sl])
                pn_t.append(pn)
            for n2 in range(ns):
                rrow = row_pool.tile([1, FD], F32, tag=f"rr{n2}")
                nc.vector.reciprocal_approx_fast(out=rrow[:], in_=pn_t[n2][:])
                rrows.append(rrow)
            for n2 in range(ns):
                srow = row_pool.tile([1, FD], CT, tag=f"sr{n2}")
                nc.scalar.activation(srow[:], rrows[n2][:], AF.Sqrt)
                srows.append(srow)

            # --- v squares half 0 + partial sums (DVE, ahead of the tT muls
            # so the vn2 column matmuls are never blocked)
            squares(vt_sb, vsq_sb, 0)
            chunk_sum(vsq_sb, vss_sb, 0)

            # --- proj-v second half, with the two 1/tn replicate matmuls
            # interleaved into the k-stream (avoids PSUM WAR stalls)
            pv1 = [
                ps_mm.tile([P, FD], F32, tag="ps_mm", name=f"pj1{m}")
                for m in range(mh)
            ]
            prs = []
            for k in range(kv):
                for m in range(mh):
                    nc.tensor.matmul(
                        pv1[m][:],
                        wv_k[k][:, ts(m, P)],
                        vis_t[k][:, ds(FD, FD)],
                        start=(k == 0),
                        stop=(k == kv - 1),
                    )
                if k < ns:  # after k=0 and k=1 groups: one repl matmul each
                    pr = ps_repl.tile([P, FD], F32, tag="ps_repl", name=f"pr{k}")
                    nc.tensor.matmul(pr[:], ones_row, srows[k][:])
                    prs.append(pr)
            proj_act(pv1, bv_sb, vt_sb, 1)

            # --- fold 1/tn into tT straight from the replicated PSUM (DVE)
            for n2 in range(ns):
                sl = ds(n2 * FD, FD)
                for m in range(mh):
                    nc.vector.tensor_mul(
                        tt_sb[:, m, sl], tt_sb[:, m, sl], prs[n2][:]
                    )

            squares(vt_sb, vsq_sb, 1)
            chunk_sum(vsq_sb, vss_sb, 1)

            # --- dots + epilogue in two i-halves; each half preceded by its
            # 4 tiny vn2 column matmuls (stationary=vss block, moving=ones
            # column) -> approx 1/vn2 -> sqrt -> per-partition 1/vn columns
            for half in range(2):
                csl = ds(half * (ms // 2), ms // 2)
                pcol = ps_mm.tile([P, ms // 2], F32, tag="ps_mm", name=f"pc{half}")
                for sb in range(ms // 2):
                    i = half * (ms // 2) + sb
                    nc.tensor.matmul(
                        pcol[:, ds(sb, 1)], vss_sb[:, ts(i, P)], ones_col
                    )
                ctmp = col_pool.tile([P, ms // 2], F32, tag=f"ctmp{half}")
                nc.vector.reciprocal_approx_fast(out=ctmp[:], in_=pcol[:])
                nc.scalar.activation(rvn_cols[:, csl], ctmp[:], AF.Sqrt)

                for sb in range(ms // 2):
                    i = half * (ms // 2) + sb
                    out_sb = out_pool.tile([P, s], CT)
                    for jc in range(ns):
                        pd = ps_mm.tile([P, FD], F32, tag="ps_mm", name="pd")
                        for hc in range(mh):
                            nc.tensor.matmul(
                                pd[:],
                                vt_sb[:, hc, ts(i, P)],
                                tt_sb[:, hc, ds(jc * FD, FD)],
                                start=(hc == 0),
                                stop=(hc == mh - 1),
                            )
                        if jc == 0:  # split epilogues across Scalar and DVE
                            nc.scalar.activation(
                                out_sb[:, ds(jc * FD, FD)], pd[:], AF.Copy,
                                scale=rvn_cols[:, ds(i, 1)],
                            )
                        else:
                            nc.vector.tensor_scalar_mul(
                                out_sb[:, ds(jc * FD, FD)], pd[:],
                                rvn_cols[:, ds(i, 1)],
                            )
                    if b == bpc - 1 and i == ms - 1:
                        lastq = [nc.sync, nc.gpsimd, nc.scalar, nc.sync]
                        for q4 in range(4):
                            lastq[q4].dma_start(
                                out[b, ds(i * P, P), ds(q4 * FD // 2, FD // 2)],
                                out_sb[:, ds(q4 * FD // 2, FD // 2)],
                            )
                    else:
                        nc.gpsimd.dma_start(out[b, ds(i * P, P), :], out_sb[:])

    nc.compile()
    return nc


_CACHE = {}


def _get_nc():
    if "nc" not in _CACHE:
        _CACHE["nc"] = build()
    return _CACHE["nc"]


def _prep_in_maps(visual_features, text_features, Wv, bv, Wt, bt):
    import ml_dtypes

    f = np.float32
    ct = ml_dtypes.bfloat16
    wvT = np.ascontiguousarray(np.asarray(Wv, dtype=f).T).astype(ct)  # [VD, H]
    wtT = np.ascontiguousarray(np.asarray(Wt, dtype=f).T).astype(ct)  # [TD, H]
    bvp = np.ascontiguousarray(np.asarray(bv, dtype=f).reshape(H // P, P).T)
    btp = np.ascontiguousarray(np.asarray(bt, dtype=f).reshape(H // P, P).T)
    ones = np.ones((P, P), dtype=f).astype(ct)
    vis = np.asarray(visual_features, dtype=f)
    txt = np.asarray(text_features, dtype=f)
    in_maps = []
    for c in range(NCORES):
        sl = slice(c * BPC, (c + 1) * BPC)
        in_maps.append({
            "visT": np.ascontiguousarray(vis[sl].transpose(0, 2, 1)).astype(ct),
            "txtT": np.ascontiguousarray(txt[sl].transpose(0, 2, 1)).astype(ct),
            "wvT": wvT,
            "wtT": wtT,
            "bvp": bvp,
            "btp": btp,
            "ones": ones,
        })
    return in_maps


def run(inputs, trace=False, tmpdir=None):
    """Returns (full_output, BassKernelResults)."""
    from concourse.bass_utils import run_bass_kernel_spmd

    nc = _get_nc()
    in_maps = _prep_in_maps(**inputs)
    res = run_bass_kernel_spmd(
        nc, in_maps, core_ids=list(range(NCORES)), trace=trace, tmpdir=tmpdir
    )
    outp = np.concatenate(
        [np.asarray(res.results[c]["out"]) for c in range(NCORES)], axis=0
    ).astype(np.float32)
    return outp, res


def kernel(**inputs) -> np.ndarray:
    outp, _ = run(inputs, trace=False)
    return outp
